# revision 1
# baseline (speedup 1.0000x reference)
"""Bilateral filter (5x5 window, sigmaXY=sigmaZ=1) on 8 Trainium2 NeuronCores.

Math: with p = neighbor value, c = center value, both in [0,1):
    sim(p,c) = w_spatial * exp(-0.5(p-c)^2)
             = w_spatial * e^{-p^2/2} e^{pc} e^{-c^2/2}
The e^{-c^2/2} factor is common to num and den and cancels in the ratio.
Recentering z'=(p-1/2)(c-1/2):  e^{pc} = e^{z'} e^{p/2} e^{c/2} e^{-1/4};
e^{c/2} also cancels, e^{p/2} folds into the p-side field.  With
    t_k = e^{-(p-1/2)^2/2 + 1/8} (p-1/2)^k     (t_0(0)=1 -> zero-pad correct)
    S_k = (5x5 gaussian) (*) t_k               (banded matmuls on TensorE)
    e^{z'} ~= sum_k a_k z'^k  (degree-D weighted-LS fit on [-1/4,1/4])
    den = sum_k a_k (c-1/2)^k S_k
    M   = sum_k a_k (c-1/2)^k S_{k+1}
    out = 1/2 + M/den

Sharding: H dim split across 8 cores (64 rows each + 2-row halo, zero-padded
at image borders host-side).  Layout per core: transposed so W is the SBUF
partition dim: x[524 cols, 12 img, 68 rows]; 5 column chunks of 104 output
cols (108 input cols) fit the 128-partition limit.  The 5x5 conv = 5
dy-shifted PSUM-accumulated matmuls with a wx-banded stationary operand.

Dtypes: fields/weights/products fp16 (same DVE 2x / PE 1-cyc tiers as bf16,
8x finer mantissa), conv accumulation + S_0 + final sums fp32.  Measured
l2 rel err vs the fp32 reference: 2.9e-4 (maxrel 1.7e-3).

Engine split per chunk: TensorE 40 banded matmuls (conv), ScalarE
square/exp/PSUM-evacuations/+0.5, VectorE field chain + polynomial
assembly + reciprocal, GPSIMD the final M*(1/den) multiply, HWDGE DMAs.
"""

import numpy as np
from contextlib import ExitStack

import concourse.bass as bass
import concourse.bacc as bacc
import concourse.tile as tile
from concourse import mybir
from concourse.bass_utils import run_bass_kernel_spmd

F32 = mybir.dt.float32
BF16 = mybir.dt.float16  # fp16: same DVE/PE speed tiers as bf16, 8x finer mantissa
NP_BF16 = mybir.dt.np(BF16)

N_CORES = 8
NIMG = 12            # 4 batch * 3 channels
H = 512
W = 512
ROWS = 64            # output rows per core
R = ROWS + 4         # input rows per core incl halo
WPAD = 524           # 512 + 2+2 conv pad + 8 slack for 5*104 chunking
NCHUNK = 5
CH_OUT = 104         # output cols per chunk
CH_IN = CH_OUT + 4   # input cols per chunk
FREE_IN = NIMG * R       # 816
FREE_OUT = NIMG * ROWS   # 768
HALF_IMGS = NIMG // 2    # 6 -> matmul free n = 6*64 = 384

DEGREE = 2
GPSIMD_ADDS = False
GPSIMD_EM = False
GPS_QD = False
GPS_DEN = False
SPLIT_DMA_Q = True
ALT_XQ = True
DMA_PROLOGUE = False
U1_ACT = False
CMS_ACT = False
CONV_ORDER = (0, 1, 2, 3)
GPS_T3 = False
GPSIMD_MR = True
EM_F16 = False
POOL_BUFS = 4
ALPHA = {
    2: [1.0, 1.0096638869735923, 0.5134352510211865],
    3: [1.0, 1.0000253488679784, 0.5031493256393234, 0.1674467221730082],
}

_W1D = np.exp(-0.5 * np.array([4.0, 1.0, 0.0, 1.0, 4.0], dtype=np.float64)).astype(
    np.float32
)


def _build_bands() -> np.ndarray:
    """bands[q, dy, o] = wx[q-o] * wy[dy] for q-o in [0,4], else 0 (bf16)."""
    b = np.zeros((CH_IN, 5, CH_OUT), dtype=np.float32)
    for o in range(CH_OUT):
        for d in range(5):
            b[o + d, :, o] = _W1D[d] * _W1D
    return b.astype(NP_BF16)


def build_nc(degree: int = DEGREE, bench_iters: int = 1):
    al = ALPHA[degree]
    nord = degree + 2  # conv orders S_0..S_{degree+1}

    nc = bacc.Bacc("TRN2", target_bir_lowering=False)
    const_tensors = []
    for v in (-0.5, 0.125, 0.5):
        t_ = nc.alloc_sbuf_tensor(f"const-f32-{v}", [128, 1], F32)
        nc.const_aps.aps[(F32, v)] = t_.ap()
        const_tensors.append((t_, v))
    x_d = nc.dram_tensor("x", [WPAD, NIMG, R], BF16, kind="ExternalInput")
    b_d = nc.dram_tensor("bands", [CH_IN, 5, CH_OUT], BF16, kind="ExternalInput")
    y_d = nc.dram_tensor("y", [WPAD, NIMG, ROWS], F32, kind="ExternalOutput")

    with ExitStack() as ctx:
        tc = ctx.enter_context(tile.TileContext(nc))
        singles = ctx.enter_context(tc.tile_pool(name="singles", bufs=1))
        fields = ctx.enter_context(tc.tile_pool(name="fields", bufs=POOL_BUFS))
        evac = ctx.enter_context(tc.tile_pool(name="evac", bufs=POOL_BUFS))
        asm = ctx.enter_context(tc.tile_pool(name="asm", bufs=POOL_BUFS))
        psum = ctx.enter_context(tc.tile_pool(name="psum", bufs=1, space="PSUM"))

        for t_, v in const_tensors:
            nc.gpsimd.memset(t_.ap(), v)
        bands = singles.tile([CH_IN, 5, CH_OUT], BF16)
        # third DMA queue family (Activation-issued HWDGE) keeps the
        # gpsimd/SWDGE queue head free for the first center-copy loads
        nc.scalar.dma_start(out=bands, in_=b_d[:])

        def body():
            xts, xcs = [], []
            if DMA_PROLOGUE:
                for j in range(NCHUNK):
                    c0 = CH_OUT * j
                    x_t = fields.tile([CH_IN, NIMG, R], BF16, name="x_t",
                                      tag="x_t", bufs=NCHUNK)
                    xq = nc.gpsimd if (ALT_XQ and j % 2) else nc.sync
                    xq.dma_start(out=x_t, in_=x_d[c0 : c0 + CH_IN])
                    x_c = fields.tile([CH_OUT, NIMG, ROWS], BF16, name="x_c",
                                      tag="x_c", bufs=NCHUNK)
                    (nc.gpsimd if SPLIT_DMA_Q else nc.sync).dma_start(
                        out=x_c, in_=x_d[c0 + 2 : c0 + 2 + CH_OUT, :, 2 : 2 + ROWS]
                    )
                    xts.append(x_t); xcs.append(x_c)
            for j in range(NCHUNK):
                c0 = CH_OUT * j
                if DMA_PROLOGUE:
                    x_t, x_c = xts[j], xcs[j]
                else:
                    x_t = fields.tile([CH_IN, NIMG, R], BF16, name="x_t", tag="x_t")
                    xq = nc.gpsimd if (ALT_XQ and j % 2) else nc.sync
                    xq.dma_start(out=x_t, in_=x_d[c0 : c0 + CH_IN])
                    # center columns, partition-aligned copy (engine APs need
                    # 32-aligned base partitions, so pm[2:106] is not readable)
                    x_c = fields.tile([CH_OUT, NIMG, ROWS], BF16, name="x_c", tag="x_c")
                    (nc.gpsimd if SPLIT_DMA_Q else nc.sync).dma_start(
                        out=x_c, in_=x_d[c0 + 2 : c0 + 2 + CH_OUT, :, 2 : 2 + ROWS]
                    )

                # p-side fields (bf16, on the full padded tile incl. halo)
                sq = fields.tile([CH_IN, NIMG, R], BF16, name="sq", tag="sq")
                pm = fields.tile([CH_IN, NIMG, R], BF16, name="pm", tag="pm")
                nc.vector.tensor_scalar_add(pm, x_t, -0.5)
                if j == 0:
                    # chunk 0: square on DVE so the first exp isn't gated on
                    # both the DMA and the ACT table load
                    nc.vector.tensor_mul(sq, pm, pm)
                else:
                    nc.scalar.activation(
                        out=sq, in_=x_t, func=mybir.ActivationFunctionType.Square,
                        bias=-0.5, scale=1.0,
                    )
                t = [fields.tile([CH_IN, NIMG, R], BF16, name="t0", tag="t0")]
                nc.scalar.activation(
                    out=t[0], in_=sq, func=mybir.ActivationFunctionType.Exp,
                    bias=0.125, scale=-0.5,
                )
                # breadth-first powers: t1=t0*pm, t2=t0*sq, t3=t1*sq
                # (sq = (p-1/2)^2 is already pm^2)
                for k in range(1, nord):
                    tk = fields.tile([CH_IN, NIMG, R], BF16, name=f"t{k}", tag=f"t{k}")
                    eng = nc.gpsimd if (GPS_T3 and k == nord - 1) else nc.vector
                    if k < 2:
                        eng.tensor_mul(tk, t[k - 1], pm)
                    else:
                        eng.tensor_mul(tk, t[k - 2], sq)
                    t.append(tk)

                # 5x5 conv of each t_k on TensorE -> PSUM fp32, evac to SBUF
                s0e = evac.tile([CH_OUT, NIMG, ROWS], F32, name="s0e", tag="s0e")
                ske = [
                    evac.tile([CH_OUT, NIMG, ROWS], BF16, name=f"s{k}e", tag=f"s{k}e")
                    for k in range(1, nord)
                ]
                for k, h in [(k, h) for h in range(2) for k in CONV_ORDER]:
                    if True:
                        i0 = h * HALF_IMGS
                        sp = psum.tile([CH_OUT, HALF_IMGS, ROWS], F32, name=f"ps{k}{h}", tag=f"ps{k}{h}")
                        for dy in range(5):
                            nc.tensor.matmul(
                                sp,
                                bands[:, dy, :],
                                t[k][:, i0 : i0 + HALF_IMGS, dy : dy + ROWS],
                                start=(dy == 0),
                                stop=(dy == 4),
                            )
                        dst = s0e if k == 0 else ske[k - 1]
                        nc.scalar.copy(
                            out=dst[:, i0 : i0 + HALF_IMGS, :], in_=sp
                        )

                # Nested-form assembly (degree 2):
                #   den = S0 + u1*(S1 + b*c'*S2),  M = S1 + u1*(S2 + b*c'*S3)
                # with u1 = a1*c', b = a2/a1, c' = c-1/2.  Both scale factors
                # fold into dual-op tensor_scalar ops on the center copy.
                assert degree == 2
                u1 = asm.tile([CH_OUT, NIMG, ROWS], BF16, name="u1", tag="u1")
                if U1_ACT:
                    a1 = float(al[1])
                    nc.scalar.activation(
                        out=u1, in_=x_c,
                        func=mybir.ActivationFunctionType.Copy,
                        scale=a1, bias=-0.5 * a1,
                    )
                else:
                    nc.vector.tensor_scalar(
                        u1, x_c, -0.5, float(al[1]),
                        mybir.AluOpType.add, mybir.AluOpType.mult,
                    )
                cms = asm.tile([CH_OUT, NIMG, ROWS], BF16, name="cms", tag="cms")
                if CMS_ACT:
                    b_ = float(al[2] / al[1])
                    nc.scalar.activation(
                        out=cms, in_=x_c,
                        func=mybir.ActivationFunctionType.Copy,
                        scale=b_, bias=-0.5 * b_,
                    )
                else:
                    nc.vector.tensor_scalar(
                        cms, x_c, -0.5, float(al[2] / al[1]),
                        mybir.AluOpType.add, mybir.AluOpType.mult,
                    )

                qd = asm.tile([CH_OUT, NIMG, ROWS], BF16, name="qd", tag="qd")
                qm = asm.tile([CH_OUT, NIMG, ROWS], BF16, name="qm", tag="qm")
                den = asm.tile([CH_OUT, NIMG, ROWS], F32, name="den", tag="den")
                # em in fp16: M is a small correction (|M/den| <~ 0.5), so
                # fp16 rounding adds only ~2e-4 abs error but keeps the add
                # in the 2x DVE mode
                em = asm.tile([CH_OUT, NIMG, ROWS], BF16 if EM_F16 else F32, name="em", tag="em")
                rden = asm.tile([CH_OUT, NIMG, ROWS], F32, name="rden", tag="rden")
                mr = asm.tile([CH_OUT, NIMG, ROWS], F32, name="mr", tag="mr")
                out_t = asm.tile([CH_OUT, NIMG, ROWS], F32, name="out_t", tag="out_t")

                def corr_sum(s_lo, s_hi, q, pfx, sl):
                    """q[sl] = u1 * (s_lo + cms * s_hi) in fp16."""
                    w = asm.tile([CH_OUT, NIMG, ROWS], BF16,
                                 name=f"{pfx}w", tag=f"{pfx}w")
                    nc.vector.tensor_mul(w[sl], cms[sl], s_hi[sl])
                    x = asm.tile([CH_OUT, NIMG, ROWS], BF16,
                                 name=f"{pfx}x", tag=f"{pfx}x")
                    nc.vector.tensor_add(x[sl], s_lo[sl], w[sl])
                    nc.vector.tensor_mul(q[sl], u1[sl], x[sl])

                def assemble(sl, last):
                    corr_sum(ske[0], ske[1], qd, "d", sl)
                    corr_sum(ske[1], ske[2], qm, "m", sl)
                    deng = nc.gpsimd if (GPS_DEN and not last) else nc.vector
                    deng.tensor_add(den[sl], s0e[sl], qd[sl])
                    nc.vector.tensor_add(em[sl], ske[0][sl], qm[sl])
                    nc.vector.reciprocal_approx_fast(out=rden[sl], in_=den[sl])
                    if GPSIMD_MR and not last:
                        nc.gpsimd.tensor_mul(mr[sl], em[sl], rden[sl])
                        nc.scalar.add(out_t[sl], mr[sl], 0.5)
                    else:
                        nc.vector.tensor_mul(mr[sl], em[sl], rden[sl])
                        nc.vector.tensor_scalar_add(out_t[sl], mr[sl], 0.5)

                n_out = min(CH_OUT, W - c0)
                if j < NCHUNK - 1:
                    assemble(np.s_[:, :, :], False)
                    nc.sync.dma_start(
                        out=y_d[c0 + 2 : c0 + 2 + n_out], in_=out_t[:n_out]
                    )
                else:
                    # last chunk: per-half so the tail overlaps the final convs
                    for h in range(2):
                        i0 = h * HALF_IMGS
                        assemble(np.s_[:, i0 : i0 + HALF_IMGS, :], True)
                        nc.sync.dma_start(
                            out=y_d[c0 + 2 : c0 + 2 + n_out, i0 : i0 + HALF_IMGS],
                            in_=out_t[:n_out, i0 : i0 + HALF_IMGS],
                        )

        if bench_iters == 1:
            body()
        else:
            hints = (
                mybir.EngineType.PE,
                mybir.EngineType.DVE,
                mybir.EngineType.Activation,
                mybir.EngineType.SP,
            )
            with tc.For_i(0, bench_iters, 1, hint_engines=hints):
                body()

    nc.finalize()
    return nc


def _prep_inputs(X: np.ndarray):
    """Full X [4,3,512,512] fp32 -> per-core transposed/padded bf16 arrays."""
    Xr = np.ascontiguousarray(np.asarray(X, dtype=np.float32).reshape(NIMG, H, W))
    bands = _build_bands()
    in_maps = []
    for i in range(N_CORES):
        lo = ROWS * i - 2
        s0, s1 = max(0, lo), min(H, lo + R)
        P = np.zeros((NIMG, R, WPAD), dtype=np.float32)
        P[:, s0 - lo : s1 - lo, 2 : 2 + W] = Xr[:, s0:s1, :]
        xt = np.ascontiguousarray(P.transpose(2, 0, 1)).astype(NP_BF16)
        in_maps.append({"x": xt, "bands": bands})
    return in_maps


_NC_CACHE = {}


def kernel(X: np.ndarray) -> np.ndarray:
    key = (DEGREE, 1)
    if key not in _NC_CACHE:
        _NC_CACHE[key] = build_nc(DEGREE, 1)
    nc = _NC_CACHE[key]
    in_maps = _prep_inputs(X)
    res = run_bass_kernel_spmd(nc, in_maps, list(range(N_CORES)))
    out = np.empty((NIMG, H, W), dtype=np.float32)
    for i in range(N_CORES):
        yi = res.results[i]["y"]  # [WPAD, NIMG, ROWS]
        out[:, ROWS * i : ROWS * (i + 1), :] = yi[2 : 2 + W].transpose(1, 2, 0)
    return out.reshape(4, 3, H, W)

